# revision 4
# baseline (speedup 1.0000x reference)
"""Trainium2 Bass kernel for the Bayesian logistic-regression activation matrix.

Computes, for x [N, D], w_mu [D], w_log_var [D], z [NS]:
    mean  = x @ w_mu                       [N]
    var   = (x*x) @ exp(w_log_var)         [N]
    out[i, j] = sqrt(var_i) * z_j + mean_i [N, NS]

Data-parallel over 8 NeuronCores, 12500 rows per core in 5 blocks of
T=2500 (5 matmul tiles of R=500 each). Same math as kernel_v3 plus a
one-block-deep software pipeline so the PE never idles long enough for
the HAM clock gate to re-throttle it:

  iter b:  chains(b) on PE  ||  epilogue(b-1) on DVE/ACT/PE  ||
           TT squares(b+1) on DVE  ||  loads(b+1) on DMA

  - Host pre-transposes + casts x to bf16 [D, nshard]; all DMAs are
    contiguous (5000B runs). Output [NS, nshard] bf16, host transposes.
  - PE chunk-major passes: 5 back-to-back same-weight matmuls per
    chunk; mean passes in column group 0 (psum partition 0) and var
    passes in column group 1 (partition 32) of the same per-tile PSUM
    bank ping-pong on separate XBUSes (~2 MMs per 211ns). PSUM
    pending-zero marking is per-partition, so both groups use
    start=True on chunk 0 without clobbering each other.
  - Epilogue per tile: mean row cast (DVE or ACT, split to balance),
    ACT sqrt, one K=33 output matmul (B33 = [ones;0..;z] against a
    [33, R] rows tile whose unused partitions are memset once), ACT
    evict into a per-block [NS, T] buffer, one store per block.
"""

import numpy as np

N = 100000
D = 512
NS = 128
NCORES = 8
NSHARD = N // NCORES  # 12500 rows per core
P = 128  # SBUF partitions
C = D // P  # 4 chunks of the feature dim
R = 500  # rows per matmul tile; fits one PSUM bank in fp32
T = 2500  # rows per DMA block
OUT_BF16 = True  # store bf16 from device, widen to f32 on host
NROWS_BUFS = 10  # 2 blocks' worth: writers of block b never collide with
# the out-matmul readers of block b-1

_CACHE = {}


def _build_bass(nshard=NSHARD, r=R, t_blk=T):
    """Build + compile the per-core Bass module (one NEFF, SPMD on 8 cores)."""
    from contextlib import ExitStack

    import concourse.bacc as bacc
    import concourse.mybir as mybir
    import concourse.tile as tile

    f32 = mybir.dt.float32
    bf16 = mybir.dt.bfloat16
    out_dt = bf16 if OUT_BF16 else f32

    assert nshard % t_blk == 0 and t_blk % r == 0
    nblocks = nshard // t_blk
    tpb = t_blk // r  # tiles per block

    nc = bacc.Bacc("TRN2", target_bir_lowering=False, debug=False)

    xt = nc.dram_tensor("xt", [D, nshard], bf16, kind="ExternalInput").ap()
    wcols = nc.dram_tensor("wcols", [P, C], bf16, kind="ExternalInput").ap()
    ecols = nc.dram_tensor("ecols", [P, C], bf16, kind="ExternalInput").ap()
    b33 = nc.dram_tensor("b33", [33, NS], bf16, kind="ExternalInput").ap()
    out = nc.dram_tensor("out_t", [NS, nshard], out_dt, kind="ExternalOutput").ap()

    with tile.TileContext(nc) as tc, ExitStack() as ctx:
        const_pool = ctx.enter_context(tc.tile_pool(name="const", bufs=1))
        x_pool = ctx.enter_context(tc.tile_pool(name="xb", bufs=3))
        sq_pool = ctx.enter_context(tc.tile_pool(name="sq", bufs=3))
        osb_pool = ctx.enter_context(tc.tile_pool(name="osb", bufs=2))
        pmv_pool = ctx.enter_context(tc.tile_pool(name="pmv", bufs=5, space="PSUM"))
        po_pool = ctx.enter_context(tc.tile_pool(name="pout", bufs=3, space="PSUM"))

        # rows tiles first (memsets have no deps, run during DMA warmup)
        rows_tiles = []
        for k in range(NROWS_BUFS):
            rt = const_pool.tile([33, r], bf16, tag=f"rows{k}", name=f"rows{k}")
            nc.vector.memset(rt[:], 0.0)
            rows_tiles.append(rt)

        xb_tiles = [None] * nblocks
        sq_tiles = [None] * nblocks

        def issue_loads(b):
            n0b = b * t_blk
            xb_tiles[b] = x_pool.tile([P, C * t_blk], bf16, tag="xb", name=f"xb{b}")
            for c in range(C):
                nc.sync.dma_start(
                    xb_tiles[b][:, c * t_blk : (c + 1) * t_blk],
                    xt[c * P : (c + 1) * P, n0b : n0b + t_blk],
                )

        # first chunk of block 0, then consts, then the rest: the first
        # mean pass needs only chunk 0 + w
        xb_tiles[0] = x_pool.tile([P, C * t_blk], bf16, tag="xb", name="xb0")
        h = t_blk // 2
        nc.sync.dma_start(xb_tiles[0][:, 0:h], xt[0:P, 0:h])
        nc.sync.dma_start(xb_tiles[0][:, h:t_blk], xt[0:P, h:t_blk])
        w_t = const_pool.tile([P, C], bf16)
        nc.sync.dma_start(w_t[:], wcols[:])
        e_t = const_pool.tile([P, C], bf16)
        nc.sync.dma_start(e_t[:], ecols[:])
        b33_t = const_pool.tile([33, NS], bf16)
        nc.sync.dma_start(b33_t[:], b33[:])
        for c in range(1, C):
            nc.sync.dma_start(
                xb_tiles[0][:, c * t_blk : (c + 1) * t_blk],
                xt[c * P : (c + 1) * P, 0:t_blk],
            )

        def issue_squares(b):
            sq_tiles[b] = sq_pool.tile([P, C * t_blk], bf16, tag="sq", name=f"sq{b}")
            for c in range(C):
                nc.vector.tensor_mul(
                    sq_tiles[b][:, c * t_blk : (c + 1) * t_blk],
                    xb_tiles[b][:, c * t_blk : (c + 1) * t_blk],
                    xb_tiles[b][:, c * t_blk : (c + 1) * t_blk],
                )

        def issue_chains(b, pmv):
            xb_t, sq_t = xb_tiles[b], sq_tiles[b]
            for c in range(C):
                for ti in range(tpb):
                    nc.tensor.matmul(
                        pmv[ti][0:1, :],
                        w_t[:, c : c + 1],
                        xb_t[:, c * t_blk + ti * r : c * t_blk + ti * r + r],
                        start=(c == 0),
                        stop=(c == C - 1),
                        skip_group_check=True,
                    )
                for ti in range(tpb):
                    nc.tensor.matmul(
                        pmv[ti][32:33, :],
                        e_t[:, c : c + 1],
                        sq_t[:, c * t_blk + ti * r : c * t_blk + ti * r + r],
                        start=(c == 0),
                        stop=(c == C - 1),
                        tile_position=(0, 32),
                        skip_group_check=True,
                    )

        def issue_rows(b, pmv):
            # mean cast + sqrt into the persistent rows tiles, right after
            # the chains of block b: frees the pmv banks so block b+1's
            # chains never wait behind next-block TTs on the DVE queue.
            # Casts split DVE/ACT to balance the two engines.
            for ti in range(tpb):
                rows_t = rows_tiles[(b * tpb + ti) % NROWS_BUFS]
                if ti < 3:
                    nc.vector.tensor_copy(rows_t[0:1, :], pmv[ti][0:1, :])
                else:
                    nc.scalar.copy(rows_t[0:1, :], pmv[ti][0:1, :])
                nc.scalar.sqrt(rows_t[32:33, :], pmv[ti][32:33, :])

        def issue_outmms(b):
            # out[j, n] = 1*mean_n + z_j*std_n: one K=33 matmul per tile.
            # On the last block the evicts alternate DVE/ACT so the kernel
            # tail isn't a single serialized ACT chain.
            n0b = b * t_blk
            last = b == nblocks - 1
            osb_t = osb_pool.tile([NS, t_blk], out_dt, tag="osb", name=f"osb{b}")
            for ti in range(tpb):
                rows_t = rows_tiles[(b * tpb + ti) % NROWS_BUFS]
                pout = po_pool.tile([NS, r], f32, tag="pout", name=f"po{b}_{ti}")
                nc.tensor.matmul(pout[:], b33_t[:], rows_t[:], start=True, stop=True)
                if last and ti % 2 == 0:
                    nc.vector.tensor_copy(osb_t[:, ti * r : (ti + 1) * r], pout[:])
                else:
                    nc.scalar.copy(osb_t[:, ti * r : (ti + 1) * r], pout[:])
            nc.sync.dma_start(out[:, n0b : n0b + t_blk], osb_t[:])

        # software pipeline, one block deep
        issue_squares(0)
        pmv_prev = None
        for b in range(nblocks):
            if b + 1 < nblocks:
                issue_loads(b + 1)
            pmv = [
                pmv_pool.tile([33, r], f32, tag="pmv", name=f"pmv{b}_{ti}")
                for ti in range(tpb)
            ]
            issue_chains(b, pmv)
            # out-matmuls of b-1 BEFORE rows(b): their ACT evicts must
            # precede sqrt(b) on the ACT queue, else pout slot reuse
            # stalls the PE ~3us per block; rows(b) casts still land
            # ahead of TT(b+1) on DVE for the pmv-slot handoff
            if b > 0:
                issue_outmms(b - 1)
            issue_rows(b, pmv)
            if b + 1 < nblocks:
                issue_squares(b + 1)
            pmv_prev = pmv
        issue_outmms(nblocks - 1)

    nc.compile()
    return nc


def _host_consts(w_mu, w_log_var, z):
    import ml_dtypes

    bf16 = ml_dtypes.bfloat16
    e = np.exp(w_log_var.astype(np.float32))
    wcols = np.ascontiguousarray(w_mu.reshape(C, P).T).astype(bf16)
    ecols = np.ascontiguousarray(e.reshape(C, P).T).astype(bf16)
    b33 = np.zeros((33, NS), dtype=bf16)
    b33[0, :] = 1.0
    b33[32, :] = z.astype(bf16)
    return wcols, ecols, b33


def _get_nc():
    if "nc" not in _CACHE:
        _CACHE["nc"] = _build_bass()
    return _CACHE["nc"]


def kernel(x, w_mu, w_log_var, z, _trace=False, _tmpdir=None):
    import ml_dtypes
    from concourse.bass_utils import run_bass_kernel_spmd

    bf16 = ml_dtypes.bfloat16
    x = np.asarray(x, dtype=np.float32)
    w_mu = np.asarray(w_mu, dtype=np.float32)
    w_log_var = np.asarray(w_log_var, dtype=np.float32)
    z = np.asarray(z, dtype=np.float32)

    wcols, ecols, b33 = _host_consts(w_mu, w_log_var, z)

    xbf = x.astype(bf16)
    in_maps = []
    for c in range(NCORES):
        xt = np.ascontiguousarray(xbf[c * NSHARD : (c + 1) * NSHARD].T)
        in_maps.append(
            {
                "xt": xt,
                "wcols": wcols,
                "ecols": ecols,
                "b33": b33,
            }
        )

    nc = _get_nc()
    res = run_bass_kernel_spmd(
        nc,
        in_maps,
        core_ids=list(range(NCORES)),
        trace=_trace,
        tmpdir=_tmpdir,
        stitch_traces=False,
    )
    _CACHE["last_results"] = res
    outs = [r["out_t"].T.astype(np.float32) for r in res.results]
    return np.concatenate(outs, axis=0)
